# revision 11
# baseline (speedup 1.0000x reference)
"""Causal cross-attention Trainium2 kernel.

Sharding: 8 cores = 2 (batch) x 4 (head-groups of 4 heads, tensor parallel).
Each core computes its batch's attention for its 4 heads plus the partial
(row-parallel) output projection; the host sums head-group partials.

Device layout is fully transposed (Q^T/K^T = [d, seq], scores = S^T[k, q]) so
that the softmax denominator comes free as a fused ones-column in the PV
matmul, and Q/K biases become per-partition adds. No max-subtraction in
softmax: scores are ~N(0,1) (scale 1/sqrt(64) folded into Wq), so raw exp is
safe in fp32.
"""

import sys

if "/opt/trn_rl_repo" not in sys.path:
    sys.path.insert(0, "/opt/trn_rl_repo")

import numpy as np

import concourse.mybir as mybir
import concourse.tile as tile
from concourse import bacc
from concourse.bass_utils import run_bass_kernel_spmd

# problem shapes (hardcoded)
B = 2
SQ = 2048
SKV = 2048
D = 1024
H = 16
DH = 64
N_CORES = 8
HG = 4  # head groups
H_PER = H // HG  # 4 heads per core
DG = H_PER * DH  # 256 dims per core

F16 = mybir.dt.float16
F32 = mybir.dt.float32

QB = 512  # q block (free dim per matmul)
KT = 128  # kv tile (partition dim)
NQB = SQ // QB  # 4
NKT = SKV // KT  # 16
NCT = D // 128  # 8 contraction tiles for projections
NDT = DG // 128  # 2 partition tiles for the 256 head dims

_CACHE = {}


def _build():
    nc = bacc.Bacc("TRN2", target_bir_lowering=False, debug=False)

    qT_ext = nc.dram_tensor("qT", [D, SQ], F16, kind="ExternalInput")
    kvT_ext = nc.dram_tensor("kvT", [D, SKV], F16, kind="ExternalInput")
    wqT_ext = nc.dram_tensor("wqT", [D, DG], F16, kind="ExternalInput")
    wkT_ext = nc.dram_tensor("wkT", [D, DG], F16, kind="ExternalInput")
    wvT_ext = nc.dram_tensor("wvT", [D, DG], F16, kind="ExternalInput")
    woT_ext = nc.dram_tensor("woT", [DG, D], F16, kind="ExternalInput")
    bqk_ext = nc.dram_tensor("bqk", [128, 2 * NDT], F32, kind="ExternalInput")
    bv_ext = nc.dram_tensor("bv", [1, DG], F16, kind="ExternalInput")
    mask_ext = nc.dram_tensor("mask", [128, 128], F16, kind="ExternalInput")
    y_ext = nc.dram_tensor("y", [SQ, D], F32, kind="ExternalOutput")

    with tile.TileContext(nc) as tc:
        with (
            tc.tile_pool(name="res", bufs=1) as res,
            tc.tile_pool(name="pp", bufs=4) as pp,
            tc.tile_pool(name="yp", bufs=2) as yp,
            tc.tile_pool(name="rp", bufs=2) as rp,
        ):
            # ---- resident loads -------------------------------------------
            qT_s = res.tile([128, NCT, SQ], F16)
            nc.sync.dma_start(
                out=qT_s, in_=qT_ext.rearrange("(kt p) q -> p kt q", p=128)
            )
            kvT_s = res.tile([128, NCT, SKV], F16)
            nc.sync.dma_start(
                out=kvT_s, in_=kvT_ext.rearrange("(kt p) q -> p kt q", p=128)
            )
            wq_s = res.tile([128, NCT, DG], F16)
            nc.sync.dma_start(
                out=wq_s, in_=wqT_ext.rearrange("(kt p) d -> p kt d", p=128)
            )
            wk_s = res.tile([128, NCT, DG], F16)
            nc.sync.dma_start(
                out=wk_s, in_=wkT_ext.rearrange("(kt p) d -> p kt d", p=128)
            )
            wv_s = res.tile([128, NCT, DG], F16)
            nc.sync.dma_start(
                out=wv_s, in_=wvT_ext.rearrange("(kt p) d -> p kt d", p=128)
            )
            wo_s = res.tile([128, NDT, D], F16)
            nc.sync.dma_start(
                out=wo_s, in_=woT_ext.rearrange("(kt p) m -> p kt m", p=128)
            )
            bqk_s = res.tile([128, 2 * NDT], F32)
            nc.sync.dma_start(out=bqk_s, in_=bqk_ext[:, :])
            bv_s = res.tile([1, DG], F16)
            nc.sync.dma_start(out=bv_s, in_=bv_ext[:, :])
            mask_s = res.tile([128, 128], F16)
            nc.sync.dma_start(out=mask_s, in_=mask_ext[:, :])
            ones_s = res.tile([1, 128], F16)
            nc.vector.memset(ones_s, 1.0)

            # ---- projections ----------------------------------------------
            QT_s = res.tile([128, NDT, SQ], F16)
            KT_s = res.tile([128, NDT, SKV], F16)
            # V with a fused ones column per head: [kv, head, 64+1]
            V_s = res.tile([128, NKT, H_PER, DH + 1], F16)
            nc.vector.memset(V_s[:, :, :, DH : DH + 1], 1.0)

            psp_cm = tc.tile_pool(name="psp", bufs=3, space="PSUM")
            psp = psp_cm.__enter__()
            for dt in range(NDT):
                for qb in range(NQB):
                    p_q = psp.tile([128, QB], F32, tag="proj")
                    p_k = psp.tile([128, QB], F32, tag="proj")
                    for kt in range(NCT):
                        nc.tensor.matmul(
                            p_q,
                            wq_s[:, kt, 128 * dt : 128 * dt + 128],
                            qT_s[:, kt, QB * qb : QB * qb + QB],
                            start=(kt == 0),
                            stop=(kt == NCT - 1),
                        )
                    for kt in range(NCT):
                        nc.tensor.matmul(
                            p_k,
                            wk_s[:, kt, 128 * dt : 128 * dt + 128],
                            kvT_s[:, kt, QB * qb : QB * qb + QB],
                            start=(kt == 0),
                            stop=(kt == NCT - 1),
                        )
                    nc.vector.tensor_scalar_add(
                        QT_s[:, dt, QB * qb : QB * qb + QB],
                        p_q,
                        bqk_s[:, dt : dt + 1],
                    )
                    nc.vector.tensor_scalar_add(
                        KT_s[:, dt, QB * qb : QB * qb + QB],
                        p_k,
                        bqk_s[:, NDT + dt : NDT + dt + 1],
                    )

            for it in range(NKT):
                p_v = psp.tile([128, DG], F32, tag="proj")
                for kt in range(NCT):
                    nc.tensor.matmul(
                        p_v,
                        kvT_s[:, kt, KT * it : KT * it + KT],
                        wv_s[:, kt, :],
                        start=(kt == 0),
                        stop=False,
                    )
                nc.tensor.matmul(p_v, ones_s, bv_s, start=False, stop=True)
                nc.vector.tensor_copy(
                    V_s[:, it, :, 0:DH],
                    p_v.rearrange("p (h d) -> p h d", h=H_PER),
                )

            psp_cm.__exit__(None, None, None)

            # ---- attention ------------------------------------------------
            pss_cm = tc.tile_pool(name="pss", bufs=3, space="PSUM")
            pss = pss_cm.__enter__()
            pso_cm = tc.tile_pool(name="pso", bufs=2, space="PSUM")
            pso = pso_cm.__enter__()
            OT_s = res.tile([128, NDT, SQ], F16)
            for h in range(H_PER):
                p0 = DH * (h % 2)
                td = h // 2
                for qb in range(NQB):
                    o_ps = pso.tile([DH + 1, QB], F32, tag="opsum")
                    n_it = 4 * qb + 4  # k-tiles covering this q block
                    for it in range(n_it):
                        c_start = max(QB * qb, KT * it)
                        width = QB * (qb + 1) - c_start
                        s_ps = pss.tile([128, QB], F32, tag="spsum")
                        nc.tensor.matmul(
                            s_ps[:, 0:width],
                            KT_s[p0 : p0 + DH, td, KT * it : KT * it + KT],
                            QT_s[p0 : p0 + DH, td, c_start : c_start + width],
                            start=True,
                            stop=True,
                        )
                        p_t = pp.tile([128, QB], F16, tag="ptile")
                        nc.scalar.activation(
                            p_t[:, 0:width],
                            s_ps[:, 0:width],
                            mybir.ActivationFunctionType.Exp,
                        )
                        if it >= 4 * qb:  # diagonal block: causal mask
                            nc.vector.tensor_mul(
                                p_t[:, 0:128], p_t[:, 0:128], mask_s
                            )
                        nc.tensor.matmul(
                            o_ps[:, c_start - QB * qb :],
                            V_s[:, it, h, :],
                            p_t[:, 0:width],
                            start=(it == 0),
                            stop=(it == n_it - 1),
                        )
                    # normalize: divide by the fused ones-column sums
                    r_t = rp.tile([1, QB], F32, tag="recip")
                    nc.vector.reciprocal(r_t, o_ps[DH : DH + 1, :])
                    rb_t = rp.tile([DH, QB], F32, tag="rbcast")
                    nc.gpsimd.partition_broadcast(rb_t, r_t)
                    nc.vector.tensor_mul(
                        OT_s[p0 : p0 + DH, td, QB * qb : QB * qb + QB],
                        o_ps[0:DH, :],
                        rb_t,
                    )

            pso_cm.__exit__(None, None, None)
            pss_cm.__exit__(None, None, None)

            # ---- output projection (partial over this head group) ---------
            psy_cm = tc.tile_pool(name="psy", bufs=4, space="PSUM")
            psy = psy_cm.__enter__()
            for qt in range(SQ // 128):
                y_ps0 = psy.tile([128, 512], F32, tag="ypsum")
                y_ps1 = psy.tile([128, 512], F32, tag="ypsum")
                y_ps = [y_ps0, y_ps1]
                for nh in range(2):
                    for kt2 in range(NDT):
                        nc.tensor.matmul(
                            y_ps[nh],
                            OT_s[:, kt2, 128 * qt : 128 * qt + 128],
                            wo_s[:, kt2, 512 * nh : 512 * nh + 512],
                            start=(kt2 == 0),
                            stop=(kt2 == NDT - 1),
                        )
                y_sb = yp.tile([128, D], F32)
                for nh in range(2):
                    nc.vector.tensor_copy(
                        y_sb[:, 512 * nh : 512 * nh + 512], y_ps[nh]
                    )
                nc.sync.dma_start(
                    out=y_ext[128 * qt : 128 * qt + 128, :], in_=y_sb
                )
            psy_cm.__exit__(None, None, None)

    nc.finalize()
    return nc


def _get_nc():
    if "nc" not in _CACHE:
        _CACHE["nc"] = _build()
    return _CACHE["nc"]


def _prep_core_inputs(c, query, key_value, Wq, bq, Wk, bk, Wv, bv, Wo, bo):
    b = c // HG
    hg = c % HG
    hs = slice(DG * hg, DG * hg + DG)
    scale = 1.0 / np.sqrt(DH)

    bqk = np.zeros((128, 2 * NDT), np.float32)
    bq_s = (bq[hs] * scale).astype(np.float32)
    bk_s = bk[hs].astype(np.float32)
    for dt in range(NDT):
        bqk[:, dt] = bq_s[128 * dt : 128 * dt + 128]
        bqk[:, NDT + dt] = bk_s[128 * dt : 128 * dt + 128]

    kk, qq = np.meshgrid(np.arange(128), np.arange(128), indexing="ij")
    mask = (qq >= kk).astype(np.float16)

    return {
        "qT": np.ascontiguousarray(query[b].T).astype(np.float16),
        "kvT": np.ascontiguousarray(key_value[b].T).astype(np.float16),
        "wqT": np.ascontiguousarray((Wq[hs, :] * scale).T).astype(np.float16),
        "wkT": np.ascontiguousarray(Wk[hs, :].T).astype(np.float16),
        "wvT": np.ascontiguousarray(Wv[hs, :].T).astype(np.float16),
        "woT": np.ascontiguousarray(Wo[:, hs].T).astype(np.float16),
        "bqk": bqk,
        "bv": bv[hs].reshape(1, DG).astype(np.float16),
        "mask": mask,
    }


def kernel(
    query,
    key_value,
    Wq,
    bq,
    Wk,
    bk,
    Wv,
    bv,
    Wo,
    bo,
    _trace=False,
):
    query = np.asarray(query)
    key_value = np.asarray(key_value)
    args = [np.asarray(a) for a in (Wq, bq, Wk, bk, Wv, bv, Wo, bo)]

    nc = _get_nc()
    in_maps = [
        _prep_core_inputs(c, query, key_value, *args) for c in range(N_CORES)
    ]
    res = run_bass_kernel_spmd(
        nc, in_maps, list(range(N_CORES)), trace=_trace
    )

    out = np.zeros((B, SQ, D), np.float32)
    for c in range(N_CORES):
        out[c // HG] += res.results[c]["y"]
    out += args[7].astype(np.float32)  # bo
    if _trace:
        return out, res
    return out
